# revision 1
# baseline (speedup 1.0000x reference)
"""2-layer GCN (gnn_message_passing) on 8 Trainium2 NeuronCores.

Strategy (nodes partitioned across 8 cores, graph/data parallel):
  - x@W1 in bf16 on PE with host-pretransposed x tiles (feature dim on partitions).
  - Edges (+ self loops handled separately) sharded by dst core, grouped by
    128-node dst block, split into lo/hi src ranges (int16 dma_gather limit),
    tiled by 128 edges.
  - Message passing: dma_gather rows of the (AllGathered) bf16 node table,
    per-tile one-hot matrix M[e, d] = (iota==dst_local)*norm built in one DVE
    tensor_scalar op, segment-sum via PE matmul accumulation into PSUM.
  - Self loops: added per dst block from the local table via DVE (no gather).
  - Layer 2 uses linearity: out2 = (Aggregate(z)) @ W2 + b2, reusing the same
    gather/one-hot machinery on z, then a small fp32 matmul via PE transpose.
"""
import sys
sys.path.insert(0, "/opt/trn_rl_repo")
import numpy as np
import ml_dtypes

N = 50000
NPAD = 50176
NC = 8
PC = NPAD // NC          # 6272 nodes per core
B = PC // 128            # 49 dst blocks per core
KDIM = 7688
KPAD = 7808
KO = KPAD // 128         # 61
H = 200
HPAD = 256
O = 8
LO = 32768               # src < LO -> lo table view [0:32768)
HIB = NPAD - 32768       # 17408; hi view rows [HIB:NPAD), idx' = src - HIB

BF16 = ml_dtypes.bfloat16


def _preprocess(x, edge_weight, W1, b1, W2, b2, edge_index):
    x = np.asarray(x, dtype=np.float32)
    edge_weight = np.asarray(edge_weight, dtype=np.float32)
    W1 = np.asarray(W1, dtype=np.float32)
    b1 = np.asarray(b1, dtype=np.float32)
    W2 = np.asarray(W2, dtype=np.float32)
    b2 = np.asarray(b2, dtype=np.float32)
    src = np.asarray(edge_index[0], dtype=np.int64)
    dst = np.asarray(edge_index[1], dtype=np.int64)

    # --- gcn_norm (with self loops, symmetric normalization)
    deg = np.bincount(dst, weights=edge_weight.astype(np.float64), minlength=N)
    deg += 1.0  # self loop weight
    dis = np.where(deg > 0, deg ** -0.5, 0.0).astype(np.float32)
    norm = dis[src] * edge_weight * dis[dst]          # real edges
    dis2 = (dis * dis).astype(np.float32)             # self loop coefficient

    # --- shard edges by dst core, group by dst block, split lo/hi by src
    core = dst // PC
    block = (dst % PC) // 128
    dstloc128 = (dst % PC) % 128
    lohi = (src >= LO).astype(np.int64)

    # group key: core, block, lohi
    key = (core * B + block) * 2 + lohi
    order = np.argsort(key, kind="stable")
    src_s, norm_s, dl_s, key_s = src[order], norm[order], dstloc128[order], key[order]
    counts = np.bincount(key_s, minlength=NC * B * 2).reshape(NC, B, 2)
    starts = np.zeros(NC * B * 2 + 1, dtype=np.int64)
    np.cumsum(counts.ravel(), out=starts[1:])

    # tiles per (block, stream): max over cores (single SPMD program)
    T_LO = np.maximum(1, -(-counts[:, :, 0].max(axis=0) // 128))   # [B]
    T_HI = np.maximum(1, -(-counts[:, :, 1].max(axis=0) // 128))
    TT = T_LO + T_HI
    TSUM = int(TT.sum())
    col_off = np.zeros(B, dtype=np.int64)
    np.cumsum(TT[:-1], out=col_off[1:])

    # --- per-core arrays
    per_core = []
    x_pad = np.zeros((NPAD, KPAD), dtype=np.float32)
    x_pad[:N, :KDIM] = x
    W1_pad = np.zeros((KPAD, H), dtype=np.float32)
    W1_pad[:KDIM] = W1
    W1_pre = np.ascontiguousarray(
        W1_pad.reshape(KO, 128, H).transpose(1, 0, 2)).astype(BF16)
    b1_rep = np.tile(b1[None, :], (128, 1)).astype(np.float32)
    b2_rep = np.tile(b2[None, :], (128, 1)).astype(np.float32)
    W2_pre = np.zeros((128, 2, O), dtype=np.float32)
    W2_pre[:128, 0, :] = W2[:128]
    W2_pre[: H - 128, 1, :] = W2[128:H]
    iota_row = np.tile(np.arange(128, dtype=np.float32), (128, 1))
    dis2_pad = np.zeros(NPAD, dtype=np.float32)
    dis2_pad[:N] = dis2

    for r in range(NC):
        xr = x_pad[r * PC:(r + 1) * PC]
        x_pre = np.ascontiguousarray(
            xr.reshape(B, 128, KO, 128).transpose(0, 3, 2, 1)
        ).reshape(B, 128, KPAD).astype(BF16)

        idx_flat = np.zeros((TSUM * 128,), dtype=np.int64)
        dl_flat = np.zeros((TSUM * 128,), dtype=np.float32)
        nm_flat = np.zeros((TSUM * 128,), dtype=np.float32)
        for b in range(B):
            for s, Tn in ((0, int(T_LO[b])), (1, int(T_HI[b]))):
                gkey = (r * B + b) * 2 + s
                lo_i, hi_i = starts[gkey], starts[gkey + 1]
                n = hi_i - lo_i
                t0 = col_off[b] + (0 if s == 0 else int(T_LO[b]))
                o0 = t0 * 128
                ids = src_s[lo_i:hi_i] - (0 if s == 0 else HIB)
                idx_flat[o0:o0 + n] = ids
                dl_flat[o0:o0 + n] = dl_s[lo_i:hi_i]
                nm_flat[o0:o0 + n] = norm_s[lo_i:hi_i]
        # wrap idx per (block, stream) group into [128, cols] int16
        idx_cols = np.zeros((128, TSUM * 8), dtype=np.int16)
        for b in range(B):
            for s, Tn in ((0, int(T_LO[b])), (1, int(T_HI[b]))):
                t0 = col_off[b] + (0 if s == 0 else int(T_LO[b]))
                grp = idx_flat[t0 * 128:(t0 + Tn) * 128].astype(np.int16)
                w = np.tile(grp.reshape(-1, 16).T, (8, 1))   # [128, Tn*8]
                idx_cols[:, t0 * 8:(t0 + Tn) * 8] = w
        dstloc = dl_flat.reshape(TSUM, 128).T.copy()
        normv = nm_flat.reshape(TSUM, 128).T.copy()
        dis2_blk = dis2_pad[r * PC:(r + 1) * PC].reshape(B, 128).T.copy()  # [128, B]

        per_core.append({
            "x_pre": x_pre, "w1": W1_pre, "b1r": b1_rep, "b2r": b2_rep,
            "w2": W2_pre, "iota": iota_row, "idx": idx_cols,
            "dstloc": dstloc, "normv": normv, "dis2": dis2_blk,
        })

    meta = {
        "T_LO": [int(v) for v in T_LO],
        "T_HI": [int(v) for v in T_HI],
        "col_off": [int(v) for v in col_off],
        "TSUM": TSUM,
    }
    return per_core, meta


def _build_program(meta):
    import concourse.bass as bass
    import concourse.bacc as bacc
    import concourse.mybir as mybir
    import concourse.tile as tile
    from concourse.masks import make_identity

    T_LO, T_HI = meta["T_LO"], meta["T_HI"]
    col_off, TSUM = meta["col_off"], meta["TSUM"]
    TMAX = max(T_LO[b] + T_HI[b] for b in range(B))

    nc = bacc.Bacc("TRN2", target_bir_lowering=False, debug=False,
                   num_devices=NC)
    f32, bf16, i16 = mybir.dt.float32, mybir.dt.bfloat16, mybir.dt.int16

    x_d = nc.dram_tensor("x_pre", [B, 128, KPAD], bf16, kind="ExternalInput")
    w1_d = nc.dram_tensor("w1", [128, KO, H], bf16, kind="ExternalInput")
    b1_d = nc.dram_tensor("b1r", [128, H], f32, kind="ExternalInput")
    b2_d = nc.dram_tensor("b2r", [128, O], f32, kind="ExternalInput")
    w2_d = nc.dram_tensor("w2", [128, 2, O], f32, kind="ExternalInput")
    iota_d = nc.dram_tensor("iota", [128, 128], f32, kind="ExternalInput")
    idx_d = nc.dram_tensor("idx", [128, TSUM * 8], i16, kind="ExternalInput")
    dl_d = nc.dram_tensor("dstloc", [128, TSUM], f32, kind="ExternalInput")
    nm_d = nc.dram_tensor("normv", [128, TSUM], f32, kind="ExternalInput")
    d2_d = nc.dram_tensor("dis2", [128, B], f32, kind="ExternalInput")
    out_d = nc.dram_tensor("out", [PC, O], f32, kind="ExternalOutput")

    with tile.TileContext(nc) as tc:
        with tc.tile_pool(name="cons", bufs=1) as cons, \
             tc.tile_pool(name="xt", bufs=3) as xtp, \
             tc.tile_pool(name="gp", bufs=2) as gp, \
             tc.tile_pool(name="mp", bufs=4) as mp, \
             tc.tile_pool(name="wk", bufs=3) as wk, \
             tc.tile_pool(name="dram", bufs=1, space="DRAM") as dram, \
             tc.tile_pool(name="pA", bufs=2, space="PSUM") as pA, \
             tc.tile_pool(name="pG", bufs=2, space="PSUM") as pG, \
             tc.tile_pool(name="pT", bufs=2, space="PSUM") as pT, \
             tc.tile_pool(name="pO", bufs=2, space="PSUM") as pO:

            # ---- constants
            w1_sb = cons.tile([128, KO, H], bf16)
            nc.sync.dma_start(w1_sb[:], w1_d.ap())
            b1_sb = cons.tile([128, H], f32)
            nc.sync.dma_start(b1_sb[:], b1_d.ap())
            b2_sb = cons.tile([128, O], f32)
            nc.sync.dma_start(b2_sb[:], b2_d.ap())
            w2_sb = cons.tile([128, 2, O], f32)
            nc.sync.dma_start(w2_sb[:], w2_d.ap())
            iota_sb = cons.tile([128, 128], f32)
            nc.sync.dma_start(iota_sb[:], iota_d.ap())
            idx_sb = cons.tile([128, TSUM * 8], i16)
            nc.sync.dma_start(idx_sb[:], idx_d.ap())
            dl_sb = cons.tile([128, TSUM], f32)
            nc.sync.dma_start(dl_sb[:], dl_d.ap())
            nm_sb = cons.tile([128, TSUM], f32)
            nc.sync.dma_start(nm_sb[:], nm_d.ap())
            d2_sb = cons.tile([128, B], f32)
            nc.sync.dma_start(d2_sb[:], d2_d.ap())
            ident = cons.tile([128, 128], f32)
            make_identity(nc, ident[:])

            hR = dram.tile([PC, HPAD], bf16)
            zR = dram.tile([PC, HPAD], bf16)
            hfull = dram.tile([NPAD, HPAD], bf16, addr_space="Shared")
            zfull = dram.tile([NPAD, HPAD], bf16, addr_space="Shared")

            # ---- phase A: h = x @ W1  (bf16, fp32 accum)
            for b in range(B):
                xt = xtp.tile([128, KO, 128], bf16, tag="xt")
                nc.sync.dma_start(xt[:], x_d.ap()[b])
                ph = pA.tile([128, H], f32, tag="ph")
                for k in range(KO):
                    nc.tensor.matmul(ph[:], lhsT=xt[:, k, :], rhs=w1_sb[:, k, :],
                                     start=(k == 0), stop=(k == KO - 1))
                hblk = wk.tile([128, HPAD], bf16, tag="hblk")
                nc.vector.tensor_copy(hblk[:, :H], ph[:])
                nc.sync.dma_start(hR[b * 128:(b + 1) * 128, :], hblk[:])

            # ---- AllGather h
            nc.gpsimd.collective_compute(
                "AllGather", mybir.AluOpType.bypass,
                ins=[hR[:]], outs=[hfull[:]],
                replica_groups=[list(range(NC))])

            # ---- aggregation layer builder
            def aggregate(table, local_tbl, b, acc_pool):
                TLb, THb = T_LO[b], T_HI[b]
                TTb = TLb + THb
                c0 = col_off[b]
                G = gp.tile([128, TMAX, HPAD], bf16, tag="G")
                nc.gpsimd.dma_gather(
                    G[:, :TLb, :], table[0:LO, :],
                    idx_sb[:, c0 * 8:(c0 + TLb) * 8],
                    TLb * 128, TLb * 128, HPAD, single_packet=False)
                nc.gpsimd.dma_gather(
                    G[:, TLb:TTb, :], table[HIB:NPAD, :],
                    idx_sb[:, (c0 + TLb) * 8:(c0 + TTb) * 8],
                    THb * 128, THb * 128, HPAD, single_packet=False)
                acc = acc_pool.tile([128, H], f32, tag="acc")
                for t in range(TTb):
                    M = mp.tile([128, 128], bf16, tag="M")
                    nc.vector.tensor_scalar(
                        out=M[:], in0=iota_sb[:],
                        scalar1=dl_sb[:, c0 + t:c0 + t + 1],
                        scalar2=nm_sb[:, c0 + t:c0 + t + 1],
                        op0=mybir.AluOpType.is_equal,
                        op1=mybir.AluOpType.mult)
                    nc.tensor.matmul(acc[:], lhsT=M[:], rhs=G[:, t, :H],
                                     start=(t == 0), stop=(t == TTb - 1))
                # self loop: + dis2[d] * local_tbl[d]
                loc = wk.tile([128, HPAD], bf16, tag="loc")
                nc.sync.dma_start(loc[:], local_tbl[b * 128:(b + 1) * 128, :])
                selfT = wk.tile([128, H], f32, tag="selfT")
                nc.vector.tensor_scalar(
                    out=selfT[:], in0=loc[:, :H],
                    scalar1=d2_sb[:, b:b + 1], scalar2=None,
                    op0=mybir.AluOpType.mult)
                return acc, selfT

            # ---- layer 1 aggregation -> z
            for b in range(B):
                acc, selfT = aggregate(hfull, hR, b, pG)
                zsum = wk.tile([128, H], f32, tag="zsum")
                nc.vector.tensor_add(out=zsum[:], in0=acc[:], in1=selfT[:])
                nc.vector.tensor_add(out=zsum[:], in0=zsum[:], in1=b1_sb[:])
                zblk = wk.tile([128, HPAD], bf16, tag="zblk")
                nc.scalar.activation(zblk[:, :H], zsum[:],
                                     mybir.ActivationFunctionType.Relu)
                nc.sync.dma_start(zR[b * 128:(b + 1) * 128, :], zblk[:])

            # ---- AllGather z
            nc.gpsimd.collective_compute(
                "AllGather", mybir.AluOpType.bypass,
                ins=[zR[:]], outs=[zfull[:]],
                replica_groups=[list(range(NC))])

            # ---- layer 2 aggregation -> out = agg2 @ W2 + b2
            for b in range(B):
                acc, selfT = aggregate(zfull, zR, b, pG)
                agg2 = wk.tile([128, H], f32, tag="agg2")
                nc.vector.tensor_add(out=agg2[:], in0=acc[:], in1=selfT[:])
                aggT = wk.tile([128, 2, 128], f32, tag="aggT")
                for kt, (k0, kw) in enumerate(((0, 128), (128, H - 128))):
                    pt = pT.tile([128, 128], f32, tag="pt")
                    nc.tensor.transpose(pt[:kw, :], agg2[:, k0:k0 + kw], ident[:])
                    nc.vector.tensor_copy(aggT[:kw, kt, :], pt[:kw, :])
                po = pO.tile([128, O], f32, tag="po")
                nc.tensor.matmul(po[:], lhsT=aggT[:, 0, :], rhs=w2_sb[:, 0, :],
                                 start=True, stop=False)
                nc.tensor.matmul(po[:], lhsT=aggT[:H - 128, 1, :],
                                 rhs=w2_sb[:H - 128, 1, :],
                                 start=False, stop=True)
                ob = wk.tile([128, O], f32, tag="ob")
                nc.vector.tensor_add(out=ob[:], in0=po[:], in1=b2_sb[:])
                nc.sync.dma_start(out_d.ap()[b * 128:(b + 1) * 128, :], ob[:])

    nc.compile()
    return nc


_CACHE = {}


def build(inputs):
    """Preprocess + build + compile; returns (nc, in_maps, meta)."""
    per_core, meta = _preprocess(**inputs)
    nc = _build_program(meta)
    return nc, per_core, meta


def kernel(**inputs) -> np.ndarray:
    from concourse import bass_utils
    nc, per_core, meta = build(inputs)
    res = bass_utils.run_bass_kernel_spmd(nc, per_core, core_ids=list(range(NC)))
    out = np.concatenate([res.results[c]["out"] for c in range(NC)], axis=0)
    return np.ascontiguousarray(out[:N]).astype(np.float32)



# revision 2
# speedup vs baseline: 1.0344x; 1.0344x over previous
"""2-layer GCN (gnn_message_passing) on 8 Trainium2 NeuronCores.

Strategy (nodes partitioned across 8 cores, graph/data parallel):
  - x@W1 in bf16 on PE with host-pretransposed x tiles (feature dim on
    partitions).
  - Edges sharded by dst core, grouped by 128-node dst block, split into
    lo/hi src ranges (int16 dma_gather limit), tiled by 128 edges.
  - Layer 1: dma_gather rows of the AllGathered bf16 h table, segment-sum
    via PE matmul with HOST-PRECOMPUTED one-hot tiles M[e, d] = norm_e *
    (dstloc_e == d) streamed in by HWDGE DMA (keeps DVE off the SWDGE
    descriptor rings so gather descriptor generation never stalls).
  - W2 applied BEFORE the layer-2 aggregation (linearity): zw = relu(z)@W2
    is computed per block (PE transpose + 2 matmuls), so only an 8-wide
    table is AllGathered (12.8MB vs 25.7MB) and layer 2 aggregates 256B
    rows with lhsT = gathered zw tile (8 stationary cols) and rhs = M.
  - Self loops handled per dst block from local tables via DVE (no gather).
"""
import sys
sys.path.insert(0, "/opt/trn_rl_repo")
import numpy as np
import ml_dtypes

N = 50000
NPAD = 50176
NC = 8
PC = NPAD // NC          # 6272 nodes per core
B = PC // 128            # 49 dst blocks per core
KDIM = 7688
KPAD = 7808
KO = KPAD // 128         # 61
H = 200
HPAD = 256
O = 8
LO = 32768               # src < LO -> lo table view [0:32768)
HIB = NPAD - 32768       # 17408; hi view rows [HIB:NPAD), idx' = src - HIB

BF16 = ml_dtypes.bfloat16


def _preprocess(x, edge_weight, W1, b1, W2, b2, edge_index):
    x = np.asarray(x, dtype=np.float32)
    edge_weight = np.asarray(edge_weight, dtype=np.float32)
    W1 = np.asarray(W1, dtype=np.float32)
    b1 = np.asarray(b1, dtype=np.float32)
    W2 = np.asarray(W2, dtype=np.float32)
    b2 = np.asarray(b2, dtype=np.float32)
    src = np.asarray(edge_index[0], dtype=np.int64)
    dst = np.asarray(edge_index[1], dtype=np.int64)

    # --- gcn_norm (with self loops, symmetric normalization)
    deg = np.bincount(dst, weights=edge_weight.astype(np.float64), minlength=N)
    deg += 1.0  # self loop weight
    dis = np.where(deg > 0, deg ** -0.5, 0.0).astype(np.float32)
    norm = dis[src] * edge_weight * dis[dst]          # real edges
    dis2 = (dis * dis).astype(np.float32)             # self loop coefficient

    # --- shard edges by dst core, group by dst block, split lo/hi by src
    core = dst // PC
    block = (dst % PC) // 128
    dstloc128 = (dst % PC) % 128
    lohi = (src >= LO).astype(np.int64)

    # group key: core, block, lohi
    key = (core * B + block) * 2 + lohi
    order = np.argsort(key, kind="stable")
    src_s, norm_s, dl_s, key_s = src[order], norm[order], dstloc128[order], key[order]
    counts = np.bincount(key_s, minlength=NC * B * 2).reshape(NC, B, 2)
    starts = np.zeros(NC * B * 2 + 1, dtype=np.int64)
    np.cumsum(counts.ravel(), out=starts[1:])

    # tiles per (block, stream): max over cores (single SPMD program)
    T_LO = np.maximum(1, -(-counts[:, :, 0].max(axis=0) // 128))   # [B]
    T_HI = np.maximum(1, -(-counts[:, :, 1].max(axis=0) // 128))
    TT = T_LO + T_HI
    TSUM = int(TT.sum())
    col_off = np.zeros(B, dtype=np.int64)
    np.cumsum(TT[:-1], out=col_off[1:])

    # --- per-core arrays
    per_core = []
    x_pad = np.zeros((NPAD, KPAD), dtype=np.float32)
    x_pad[:N, :KDIM] = x
    W1_pad = np.zeros((KPAD, H), dtype=np.float32)
    W1_pad[:KDIM] = W1
    W1_pre = np.ascontiguousarray(
        W1_pad.reshape(KO, 128, H).transpose(1, 0, 2)).astype(BF16)
    b1_rep = np.tile(b1[None, :], (128, 1)).astype(np.float32)
    b2_rep = np.tile(b2[None, :], (128, 1)).astype(np.float32)
    W2_pre = np.zeros((128, 2, O), dtype=np.float32)
    W2_pre[:128, 0, :] = W2[:128]
    W2_pre[: H - 128, 1, :] = W2[128:H]
    iota_row = np.tile(np.arange(128, dtype=np.float32), (128, 1))
    dis2_pad = np.zeros(NPAD, dtype=np.float32)
    dis2_pad[:N] = dis2

    for r in range(NC):
        xr = x_pad[r * PC:(r + 1) * PC]
        x_pre = np.ascontiguousarray(
            xr.reshape(B, 128, KO, 128).transpose(0, 3, 2, 1)
        ).reshape(B, 128, KPAD).astype(BF16)

        idx_flat = np.zeros((TSUM * 128,), dtype=np.int64)
        dl_flat = np.zeros((TSUM * 128,), dtype=np.float32)
        nm_flat = np.zeros((TSUM * 128,), dtype=np.float32)
        for b in range(B):
            for s, Tn in ((0, int(T_LO[b])), (1, int(T_HI[b]))):
                gkey = (r * B + b) * 2 + s
                lo_i, hi_i = starts[gkey], starts[gkey + 1]
                n = hi_i - lo_i
                t0 = col_off[b] + (0 if s == 0 else int(T_LO[b]))
                o0 = t0 * 128
                ids = src_s[lo_i:hi_i] - (0 if s == 0 else HIB)
                idx_flat[o0:o0 + n] = ids
                dl_flat[o0:o0 + n] = dl_s[lo_i:hi_i]
                nm_flat[o0:o0 + n] = norm_s[lo_i:hi_i]
        # wrap idx per (block, stream) group into [128, cols] int16
        idx_cols = np.zeros((128, TSUM * 8), dtype=np.int16)
        for b in range(B):
            for s, Tn in ((0, int(T_LO[b])), (1, int(T_HI[b]))):
                t0 = col_off[b] + (0 if s == 0 else int(T_LO[b]))
                grp = idx_flat[t0 * 128:(t0 + Tn) * 128].astype(np.int16)
                w = np.tile(grp.reshape(-1, 16).T, (8, 1))   # [128, Tn*8]
                idx_cols[:, t0 * 8:(t0 + Tn) * 8] = w
        dis2_blk = dis2_pad[r * PC:(r + 1) * PC].reshape(B, 128).T.copy()  # [128, B]

        # host-built one-hot: mall[lane, tile, d] = nm * (dstloc == d)
        mall = np.zeros((TSUM * 128, 128), np.float32)
        mall[np.arange(TSUM * 128), dl_flat.astype(np.int64)] = nm_flat
        mall = np.ascontiguousarray(
            mall.reshape(TSUM, 128, 128).transpose(1, 0, 2)).astype(BF16)

        per_core.append({
            "x_pre": x_pre, "w1": W1_pre, "b1r": b1_rep, "b2r": b2_rep,
            "w2": W2_pre, "iota": iota_row, "idx": idx_cols,
            "dis2": dis2_blk, "mall": mall,
        })

    meta = {
        "T_LO": [int(v) for v in T_LO],
        "T_HI": [int(v) for v in T_HI],
        "col_off": [int(v) for v in col_off],
        "TSUM": TSUM,
    }
    return per_core, meta


def _build_program(meta):
    import concourse.bass as bass
    import concourse.bacc as bacc
    import concourse.mybir as mybir
    import concourse.tile as tile
    from concourse.masks import make_identity

    T_LO, T_HI = meta["T_LO"], meta["T_HI"]
    col_off, TSUM = meta["col_off"], meta["TSUM"]
    TMAX = max(T_LO[b] + T_HI[b] for b in range(B))
    GB = 2

    nc = bacc.Bacc("TRN2", target_bir_lowering=False, debug=False,
                   num_devices=NC)
    f32, bf16, i16 = mybir.dt.float32, mybir.dt.bfloat16, mybir.dt.int16

    x_d = nc.dram_tensor("x_pre", [B, 128, KPAD], bf16, kind="ExternalInput")
    w1_d = nc.dram_tensor("w1", [128, KO, H], bf16, kind="ExternalInput")
    b1_d = nc.dram_tensor("b1r", [128, H], f32, kind="ExternalInput")
    b2_d = nc.dram_tensor("b2r", [128, O], f32, kind="ExternalInput")
    w2_d = nc.dram_tensor("w2", [128, 2, O], f32, kind="ExternalInput")
    iota_d = nc.dram_tensor("iota", [128, 128], f32, kind="ExternalInput")
    idx_d = nc.dram_tensor("idx", [128, TSUM * 8], i16, kind="ExternalInput")
    d2_d = nc.dram_tensor("dis2", [128, B], f32, kind="ExternalInput")
    m_d = nc.dram_tensor("mall", [128, TSUM, 128], bf16, kind="ExternalInput")
    out_d = nc.dram_tensor("out", [PC, O], f32, kind="ExternalOutput")

    with tile.TileContext(nc) as tc:
        with tc.tile_pool(name="cons", bufs=1) as cons, \
             tc.tile_pool(name="xt", bufs=2) as xtp, \
             tc.tile_pool(name="gp", bufs=GB) as gp, \
             tc.tile_pool(name="g8", bufs=GB) as g8p, \
             tc.tile_pool(name="mb", bufs=2) as mbp, \
             tc.tile_pool(name="wk", bufs=3) as wk, \
             tc.tile_pool(name="dram", bufs=1, space="DRAM") as dram, \
             tc.tile_pool(name="pA", bufs=2, space="PSUM") as pA, \
             tc.tile_pool(name="pG", bufs=2, space="PSUM") as pG, \
             tc.tile_pool(name="pT", bufs=1, space="PSUM") as pT, \
             tc.tile_pool(name="pO", bufs=1, space="PSUM") as pO:

            # ---- constants
            w1_sb = cons.tile([128, KO, H], bf16)
            nc.sync.dma_start(w1_sb[:], w1_d.ap())
            b1_sb = cons.tile([128, H], f32)
            nc.sync.dma_start(b1_sb[:], b1_d.ap())
            b2_sb = cons.tile([128, O], f32)
            nc.sync.dma_start(b2_sb[:], b2_d.ap())
            w2_sb = cons.tile([128, 2, O], f32)
            nc.sync.dma_start(w2_sb[:], w2_d.ap())
            iota_sb = cons.tile([128, 128], f32)
            nc.sync.dma_start(iota_sb[:], iota_d.ap())
            idx_sb = cons.tile([128, TSUM * 8], i16)
            nc.sync.dma_start(idx_sb[:], idx_d.ap())
            d2_sb = cons.tile([128, B], f32)
            nc.sync.dma_start(d2_sb[:], d2_d.ap())
            ident = cons.tile([128, 128], f32)
            make_identity(nc, ident[:])

            hR = dram.tile([PC, HPAD], bf16)
            zwR = dram.tile([PC, 128], bf16)
            hfull = dram.tile([NPAD, HPAD], bf16, addr_space="Shared")
            zwfull = dram.tile([NPAD, 128], bf16, addr_space="Shared")

            # ---- phase A: h = x @ W1  (bf16, fp32 accum)
            for b in range(B):
                xt = xtp.tile([128, KO, 128], bf16, tag="xt")
                nc.sync.dma_start(xt[:], x_d.ap()[b])
                ph = pA.tile([128, H], f32, tag="ph")
                for k in range(KO):
                    nc.tensor.matmul(ph[:], lhsT=xt[:, k, :], rhs=w1_sb[:, k, :],
                                     start=(k == 0), stop=(k == KO - 1))
                hblk = wk.tile([128, HPAD], bf16, tag="hblk")
                nc.vector.tensor_copy(hblk[:, :H], ph[:])
                nc.sync.dma_start(hR[b * 128:(b + 1) * 128, :], hblk[:])

            # ---- AllGather h
            nc.gpsimd.collective_compute(
                "AllGather", mybir.AluOpType.bypass,
                ins=[hR[:]], outs=[hfull[:]],
                replica_groups=[list(range(NC))])

            # ---- layer 1: aggregate h -> z; zw = relu(z)@W2 per block
            for b in range(B):
                TLb, THb = T_LO[b], T_HI[b]
                TTb = TLb + THb
                c0 = col_off[b]
                G = gp.tile([128, TMAX, HPAD], bf16, tag="G")
                nc.gpsimd.dma_gather(
                    G[:, :TLb, :], hfull[0:LO, :],
                    idx_sb[:, c0 * 8:(c0 + TLb) * 8],
                    TLb * 128, TLb * 128, HPAD, single_packet=False)
                nc.gpsimd.dma_gather(
                    G[:, TLb:TTb, :], hfull[HIB:NPAD, :],
                    idx_sb[:, (c0 + TLb) * 8:(c0 + TTb) * 8],
                    THb * 128, THb * 128, HPAD, single_packet=False)
                acc = pG.tile([128, H], f32, tag="acc")
                Mb = mbp.tile([128, TTb, 128], bf16, tag="Mb")
                nc.sync.dma_start(Mb[:], m_d.ap()[:, c0:c0 + TTb, :])
                for t in range(TTb):
                    nc.tensor.matmul(acc[:], lhsT=Mb[:, t, :], rhs=G[:, t, :H],
                                     start=(t == 0), stop=(t == TTb - 1))
                # self loop + bias + relu
                loc = wk.tile([128, HPAD], bf16, tag="loc")
                nc.sync.dma_start(loc[:], hR[b * 128:(b + 1) * 128, :])
                selfT = wk.tile([128, H], f32, tag="selfT")
                nc.vector.tensor_scalar(
                    out=selfT[:], in0=loc[:, :H],
                    scalar1=d2_sb[:, b:b + 1], scalar2=None,
                    op0=mybir.AluOpType.mult)
                zsum = wk.tile([128, H], f32, tag="zsum")
                nc.vector.tensor_add(out=zsum[:], in0=acc[:], in1=selfT[:])
                nc.vector.tensor_add(out=zsum[:], in0=zsum[:], in1=b1_sb[:])
                zf = wk.tile([128, H], f32, tag="zf")
                nc.scalar.activation(zf[:], zsum[:],
                                     mybir.ActivationFunctionType.Relu)
                # zw = z @ W2  (transpose z, 2 matmuls)
                zT = wk.tile([128, 2, 128], f32, tag="zT")
                for kt, (k0, kw) in enumerate(((0, 128), (128, H - 128))):
                    pt = pT.tile([128, 128], f32, tag="pt")
                    nc.tensor.transpose(pt[:kw, :], zf[:, k0:k0 + kw], ident[:])
                    nc.vector.tensor_copy(zT[:kw, kt, :], pt[:kw, :])
                po = pO.tile([128, O], f32, tag="po")
                nc.tensor.matmul(po[:], lhsT=zT[:, 0, :], rhs=w2_sb[:, 0, :],
                                 start=True, stop=False)
                nc.tensor.matmul(po[:], lhsT=zT[:H - 128, 1, :],
                                 rhs=w2_sb[:H - 128, 1, :],
                                 start=False, stop=True)
                zwblk = wk.tile([128, 128], bf16, tag="zwblk")
                nc.vector.tensor_copy(zwblk[:, :O], po[:])
                nc.sync.dma_start(zwR[b * 128:(b + 1) * 128, :], zwblk[:])

            # ---- AllGather zw (small: 12.8MB total)
            nc.gpsimd.collective_compute(
                "AllGather", mybir.AluOpType.bypass,
                ins=[zwR[:]], outs=[zwfull[:]],
                replica_groups=[list(range(NC))])

            # ---- layer 2: aggregate zw (8 cols) -> out
            for b in range(B):
                TLb, THb = T_LO[b], T_HI[b]
                TTb = TLb + THb
                c0 = col_off[b]
                G8 = g8p.tile([128, TMAX, 128], bf16, tag="G8")
                nc.gpsimd.dma_gather(
                    G8[:, :TLb, :], zwfull[0:LO, :],
                    idx_sb[:, c0 * 8:(c0 + TLb) * 8],
                    TLb * 128, TLb * 128, 128, single_packet=False)
                nc.gpsimd.dma_gather(
                    G8[:, TLb:TTb, :], zwfull[HIB:NPAD, :],
                    idx_sb[:, (c0 + TLb) * 8:(c0 + TTb) * 8],
                    THb * 128, THb * 128, 128, single_packet=False)
                accT8 = pO.tile([O, 128], f32, tag="accT8")
                Mb = mbp.tile([128, TTb, 128], bf16, tag="Mb2")
                nc.sync.dma_start(Mb[:], m_d.ap()[:, c0:c0 + TTb, :])
                for t in range(TTb):
                    nc.tensor.matmul(accT8[:], lhsT=G8[:, t, :O], rhs=Mb[:, t, :],
                                     start=(t == 0), stop=(t == TTb - 1))
                # accT8 [8, 128d] -> transpose -> [128d, 8]
                a8 = wk.tile([128, 128], f32, tag="a8")
                nc.vector.tensor_copy(a8[:], iota_sb[:])
                nc.vector.tensor_copy(a8[:O, :], accT8[:])
                pt8 = pT.tile([128, 128], f32, tag="pt8")
                nc.tensor.transpose(pt8[:, :], a8[:, :], ident[:])
                # self loop: dis2[d] * zw_local[d] + b2
                locw = wk.tile([128, 128], bf16, tag="locw")
                nc.sync.dma_start(locw[:], zwR[b * 128:(b + 1) * 128, :])
                selfw = wk.tile([128, O], f32, tag="selfw")
                nc.vector.tensor_scalar(
                    out=selfw[:], in0=locw[:, :O],
                    scalar1=d2_sb[:, b:b + 1], scalar2=None,
                    op0=mybir.AluOpType.mult)
                ob = wk.tile([128, O], f32, tag="ob")
                nc.vector.tensor_add(out=ob[:], in0=pt8[:, :O], in1=selfw[:])
                nc.vector.tensor_add(out=ob[:], in0=ob[:], in1=b2_sb[:])
                nc.sync.dma_start(out_d.ap()[b * 128:(b + 1) * 128, :], ob[:])

    nc.compile()
    return nc


def build(inputs):
    """Preprocess + build + compile; returns (nc, in_maps, meta)."""
    per_core, meta = _preprocess(**inputs)
    nc = _build_program(meta)
    return nc, per_core, meta


def kernel(**inputs) -> np.ndarray:
    from concourse import bass_utils
    nc, per_core, meta = build(inputs)
    res = bass_utils.run_bass_kernel_spmd(nc, per_core, core_ids=list(range(NC)))
    out = np.concatenate([res.results[c]["out"] for c in range(NC)], axis=0)
    return np.ascontiguousarray(out[:N]).astype(np.float32)
